# revision 17
# baseline (speedup 1.0000x reference)
"""Trainium2 Bass kernel for nn_EnhancedQuantumLayer (6-qubit circuit, B=32768).

Reduction: the circuit is AngleEmbedding (per-sample RX product state) followed
by a batch-independent 64x64 unitary U (StronglyEntanglingLayers + CNOT rings,
function of `weights` only), then per-qubit PauliZ expectations.

Per sample b:
    m_b   = kron_q [cos(a_q/2), sin(a_q/2)]           (real 64-vec, a = x*scale)
    A_b   = [Re(Cc^T) ; Im(Cc^T)] m_b                 (128-vec; Cc folds the
                                                       (-i)^popcount embedding
                                                       phases into U)
    EV_bq = sum_p sgn2[p,q] * A_b[p]^2                (signs of PauliZ)

Device work per core (4096 samples), bf16 matmul pipeline:
    SP    1 input DMA (angles f32, 426KB) + 1 bf16 output DMA (156KB);
          weights (bf16, 34KB) DMA'd once outside the rep loop
    ACT   1 fused Sin (832 cols, bf16 out) + 4 Square (PSUM->bf16 SBUF);
          sin/square/copy share one activation table (trig_and_small)
    Pool  5 fused broadcast-AP kron multiplies building M in the 32x32
          block-swizzled layout (SBUF only; Pool cannot touch PSUM)
    DVE   1 bf16 StreamTranspose (= M_T, basis on partitions) + 1 fat
          (38,2048) PSUM->SBUF bf16 EV copy
    PE    8 bf16 64->128 main matmuls + 8 bf16 sign matmuls (512-col
          chunks; PSUM-bank limit), sign outputs packed at partition
          bases 0/32 of a dedicated evp bank pair

All SBUF buffers double-buffered; PSUM: 2 A-slabs (2 banks each) + evp
(4 banks). Host does the tiny weights->matrix precompute, the lane
permutation/bias pre-add of the input, and the inverse permutation of
the bf16 output.
"""
import math
from contextlib import ExitStack

import numpy as np
import ml_dtypes

import concourse.bass as bass
import concourse.mybir as mybir
from concourse.bass_utils import run_bass_kernel_spmd

F32 = mybir.dt.float32
BF16 = mybir.dt.bfloat16
NQ = 6
NL = 6
B = 32768
NCORES = 8
BC = B // NCORES          # 4096 samples per core
NSB = 64                  # angle blocks per lane (s, t', p_hi)


# ---------------------------------------------------------------- host precompute
def _host_matrices(weights):
    """(CcPacked (64,128) f32, SgnZ2 (128,6) f32) from weights (6,6,3)."""
    w = np.asarray(weights, dtype=np.float64)
    phi, theta, omega = w[..., 0], w[..., 1], w[..., 2]
    ct, st = np.cos(0.5 * theta), np.sin(0.5 * theta)
    em = np.exp(-0.5j * (phi + omega))
    ep = np.exp(0.5j * (phi + omega))
    epm = np.exp(0.5j * (phi - omega))
    emp = np.exp(-0.5j * (phi - omega))

    state = np.eye(64, dtype=np.complex128).reshape((64,) + (2,) * NQ)

    def apply_1q(state, U, q):
        ax = q + 1
        s = np.moveaxis(state, ax, -1)
        s = np.einsum('ij,...j->...i', U, s)
        return np.moveaxis(s, -1, ax)

    def cnot(state, c, t):
        ca, ta = c + 1, t + 1
        s0 = np.take(state, 0, axis=ca)
        s1 = np.take(state, 1, axis=ca)
        t_in = ta - 1 if ta > ca else ta
        s1 = np.flip(s1, axis=t_in)
        return np.stack([s0, s1], axis=ca)

    for l in range(NL):
        for q in range(NQ):
            U = np.array([
                [em[l, q] * ct[l, q], -epm[l, q] * st[l, q]],
                [emp[l, q] * st[l, q], ep[l, q] * ct[l, q]],
            ])
            state = apply_1q(state, U, q)
        r = (l % (NQ - 1)) + 1
        for q in range(NQ):
            state = cnot(state, q, (q + r) % NQ)

    stateF = state.reshape(64, 64)            # [in_e, out_o] = U[o, e]
    e = np.arange(64)
    pc = np.array([bin(v).count('1') for v in e])
    phase = (-1j) ** pc                       # (-i)^popcount: RX embedding phases
    Cc = phase[:, None] * stateF              # (64_in, 64_out)

    # device row j has qubit q at bit q; reference index e has qubit 0 as MSB
    bitrev = np.array([int(format(j, '06b')[::-1], 2) for j in range(64)])
    Cdev = Cc[bitrev, :]

    ccpacked = np.concatenate([Cdev.real, Cdev.imag], axis=1)   # (64, 128)

    o = np.arange(64)
    z = np.stack([1.0 - 2.0 * ((o >> (5 - q)) & 1) for q in range(NQ)], axis=1)
    sgn2 = np.concatenate([z, z], axis=0)                        # (128, 6)
    return ccpacked.astype(np.float32), sgn2.astype(np.float32)


def _lane_sample_index():
    """SL[L, sb]: sample_local for lane L, angle-block sb."""
    L = np.arange(128)
    h, jh, pl = L >> 6, (L >> 5) & 1, L & 31
    sb = np.arange(64)
    s, tp, p_hi = sb >> 4, (sb >> 2) & 3, sb & 3
    return (1024 * p_hi[None, :] + 32 * pl[:, None]
            + 8 * s[None, :] + 2 * tp[None, :] + h[:, None])


def _out_sample_index():
    """SAMP[h, m]: sample_local for group h, M_T column m (m = 32*sb + pl)."""
    h = np.arange(2)[:, None]
    m = np.arange(2048)[None, :]
    sb, pl = m >> 5, m & 31
    s, tp, p_hi = sb >> 4, (sb >> 2) & 3, sb & 3
    return 1024 * p_hi + 32 * pl + 8 * s + 2 * tp + h


_SL = _lane_sample_index()
_SAMP = _out_sample_index()


# ---------------------------------------------------------------- device program
def _build_bass(reps=1):
    nc = bass.Bass()
    xin = nc.dram_tensor("xin", [128, 832], F32, kind="ExternalInput")
    win = nc.dram_tensor("win", [128, 134], BF16, kind="ExternalInput")
    out = nc.dram_tensor("out", [38, 2048], BF16, kind="ExternalOutput")

    ctx = ExitStack()
    with ctx:
        sb2 = lambda nm, shape, dt: [
            ctx.enter_context(nc.sbuf_tensor(f"{nm}{p}", shape, dt))
            for p in range(2)]
        ps = lambda nm, shape: ctx.enter_context(nc.psum_tensor(nm, shape, F32))

        xt = [ctx.enter_context(nc.sbuf_tensor(f"xt{p}", [128, 832], F32))
              for p in range(4)]
        scs = [ctx.enter_context(nc.sbuf_tensor(f"scs{p}", [128, 832], BF16))
               for p in range(4)]
        k1b = sb2("k1b", [128, 256], F32)
        k2b = sb2("k2b", [128, 256], F32)
        k3b = sb2("k3b", [128, 128], BF16)
        m12b = sb2("m12b", [128, 1024], BF16)
        mswz = sb2("mswz", [128, 2048], BF16)
        mtall = sb2("mtall", [128, 2048], BF16)
        sq = sb2("sq", [128, 4096], BF16)
        stg = [ctx.enter_context(nc.sbuf_tensor(f"stg{p}", [38, 2048], BF16))
               for p in range(4)]
        wt = ctx.enter_context(nc.sbuf_tensor("wt", [128, 134], BF16))

        slab = [ps("slabA", [128, 1024]), ps("slabB", [128, 1024])]
        evp = ps("evp", [38, 2048])

        sem = lambda nm: ctx.enter_context(nc.semaphore(name=nm))
        Sd, Sa, Sk, Sv, Sp, So = (sem("Sd"), sem("Sa"), sem("Sk"),
                                  sem("Sv"), sem("Sp"), sem("So"))

        block = ctx.enter_context(nc.Block())

        def hsq(P, q):
            return (scs[P].ap()[:, 0:768]
                    .rearrange("p (hf sb q) -> p sb hf q", hf=2, q=NQ)
                    [:, :, :, q:q + 1])

        # --- DVE emission order (Sv positions), computed up front
        dve_seq = []
        for i in range(reps):
            if i >= 2:
                dve_seq += [("L", i - 2), ("R", i - 2)]
            else:
                dve_seq += [("dL", i), ("dR", i)]
            dve_seq += [("K5", i), ("T", i)]
        for r in range(max(0, reps - 2), reps):
            dve_seq += [("L", r), ("R", r)]
        pos = {key: idx + 1 for idx, key in enumerate(dve_seq)}

        @block.sync
        def _(sync):
            sync.dma_start(out=wt.ap()[:, :], in_=win[:, :]).then_inc(Sd, 16)
            done = set()

            def out_dma(r):
                o = sync.dma_start(out=out[:, :], in_=stg[r % 4].ap()[:, :])
                o._wait_ge(Sv, pos[("R", r)]).then_inc(So, 16)
                done.add(r)

            for i in range(reps):
                d = sync.dma_start(out=xt[i % 4].ap()[:, :], in_=xin[:, :])
                if i >= 4:
                    d._wait_ge(Sk, 4 * i - 13)  # kron3(i-4): scs+xt free
                d.then_inc(Sd, 16)
                if i >= 3:
                    out_dma(i - 3)
            for r in range(reps):
                if r not in done:
                    out_dma(r)
            sync.wait_ge(So, 16 * reps)

        # --- ACT emission order (Sa positions): Sin runs 3 reps ahead
        act_seq = [("sin", min(k, reps - 1)) for k in range(min(3, reps))]
        act_seq = [("sin", k) for k in range(min(3, reps))]
        for i in range(reps):
            act_seq += [("sq", i, j) for j in range(4)]
            if i + 3 < reps:
                act_seq.append(("sin", i + 3))
        apos = {key: idx + 1 for idx, key in enumerate(act_seq)}

        @block.scalar
        def _(scalar):
            sfn = mybir.ActivationFunctionType.Sin
            sqf = mybir.ActivationFunctionType.Square
            for key in act_seq:
                if key[0] == "sin":
                    i = key[1]
                    a = nc.scalar.activation(scs[i % 4].ap()[:, :],
                                             xt[i % 4].ap()[:, :], sfn)
                    a._wait_ge(Sd, 16 * (i + 2)).then_inc(Sa, 1)
                else:
                    _, i, j = key
                    thr = {0: 2, 1: 4, 2: 6, 3: 10}[j]
                    q_ = nc.scalar.activation(
                        sq[i % 2].ap()[:, 1024 * j:1024 * j + 1024],
                        slab[j % 2].ap()[:, :], sqf)
                    q_._wait_ge(Sp, 16 * i + thr).then_inc(Sa, 1)

        @block.gpsimd
        def _(g):
            # Sk: 4/rep (4 krons; K5 lives on DVE)
            for i in range(reps):
                P = i % 2
                P4 = i % 4
                o1 = k1b[P].ap().rearrange("p (sb b1 b0) -> p sb b1 b0",
                                           b1=2, b0=2)
                i0 = hsq(P4, 0).squeeze(3).unsqueeze(2).broadcast_to((128, 64, 2, 2))
                i1 = hsq(P4, 1).squeeze(3).unsqueeze(3).broadcast_to((128, 64, 2, 2))
                t = nc.gpsimd.tensor_mul(o1, i0, i1)
                t._wait_ge(Sa, apos[("sin", i)]).then_inc(Sk, 1)
                o2 = k2b[P].ap().rearrange("p (sb b3 b2) -> p sb b3 b2",
                                           b3=2, b2=2)
                i0 = hsq(P4, 2).squeeze(3).unsqueeze(2).broadcast_to((128, 64, 2, 2))
                i1 = hsq(P4, 3).squeeze(3).unsqueeze(3).broadcast_to((128, 64, 2, 2))
                t = nc.gpsimd.tensor_mul(o2, i0, i1)
                if i >= 2:
                    t._wait_ge(Sv, pos[("K5", i - 2)])  # m12b/k3b[P] free
                t.then_inc(Sk, 1)
                o3 = k3b[P].ap().rearrange("p (sb b4) -> p sb b4", b4=2)
                i0 = hsq(P4, 4).squeeze(3)
                i1 = (scs[P4].ap()[:, 768:832]
                      .rearrange("p (sb o) -> p sb o", o=1)
                      .broadcast_to((128, 64, 2)))
                nc.gpsimd.tensor_mul(o3, i0, i1).then_inc(Sk, 1)
                om = m12b[P].ap().rearrange("p (sb b32 b10) -> p sb b32 b10",
                                            b32=4, b10=4)
                i0 = (k1b[P].ap().rearrange("p (sb w) -> p sb w", w=4)
                      .unsqueeze(2).broadcast_to((128, 64, 4, 4)))
                i1 = (k2b[P].ap().rearrange("p (sb w) -> p sb w", w=4)
                      .unsqueeze(3).broadcast_to((128, 64, 4, 4)))
                nc.gpsimd.tensor_mul(om, i0, i1).then_inc(Sk, 1)

        @block.vector
        def _(v):
            for key in dve_seq:
                kind, r = key
                P, Q = r % 2, r % 4
                if kind == "L":
                    c = nc.vector.tensor_copy(stg[Q].ap()[:, 0:1024],
                                              evp.ap()[:, 0:1024])
                    c._wait_ge(Sp, 16 * r + 12).then_inc(Sv, 1)
                elif kind == "R":
                    c = nc.vector.tensor_copy(stg[Q].ap()[:, 1024:2048],
                                              evp.ap()[:, 1024:2048])
                    c._wait_ge(Sp, 16 * r + 16).then_inc(Sv, 1)
                elif kind in ("dL", "dR"):
                    dm = nc.vector.tensor_copy(stg[Q].ap()[0:1, 0:4],
                                               stg[Q].ap()[0:1, 4:8])
                    dm.then_inc(Sv, 1)
                elif kind == "K5":
                    k5 = nc.vector.tensor_mul(
                        mswz[P].ap().rearrange("p (sb b4 w) -> p sb b4 w",
                                               b4=2, w=16),
                        (m12b[P].ap().rearrange("p (sb w) -> p sb w", w=16)
                         .unsqueeze(2).broadcast_to((128, 64, 2, 16))),
                        (k3b[P].ap().rearrange("p (sb b4) -> p sb b4", b4=2)
                         .unsqueeze(3).broadcast_to((128, 64, 2, 16))))
                    k5._wait_ge(Sk, 4 * r + 4).then_inc(Sv, 1)
                else:   # T
                    t = nc.vector.transpose(mtall[P].ap()[:, :],
                                            mswz[P].ap()[:, :])
                    t.then_inc(Sv, 1)

        @block.tensor
        def _(tensor):
            # Sp: 16/rep; order: mm0p mm1p mm2p q0p mm3p q1p q2p q3p
            for i in range(reps):
                P = i % 2

                def main_pair(j, wait=None, wait1=None):
                    h, half = divmod(j, 2)
                    for k in range(2):
                        mm = nc.tensor.matmul(
                            slab[j % 2].ap()[:, 512 * k:512 * k + 512],
                            wt.ap()[64 * h:64 * h + 64, 0:128],
                            mtall[P].ap()[64 * h:64 * h + 64,
                                          1024 * half + 512 * k:
                                          1024 * half + 512 * k + 512],
                            start=True, stop=True)
                        w = wait if k == 0 else wait1
                        if w is not None:
                            mm._wait_ge(*w)
                        mm.then_inc(Sp, 1)

                def sign_pair(q, wait0=None, wait1=None):
                    for k in range(2):
                        mm = nc.tensor.matmul(
                            evp.ap()[32 * (q % 2):32 * (q % 2) + NQ,
                                     1024 * (q // 2) + 512 * k:
                                     1024 * (q // 2) + 512 * k + 512],
                            wt.ap()[:, 128:134],
                            sq[P].ap()[:, 1024 * q + 512 * k:
                                       1024 * q + 512 * k + 512],
                            start=True, stop=True)
                        w = wait0 if k == 0 else wait1
                        if w is not None:
                            mm._wait_ge(*w)
                        mm.then_inc(Sp, 1)

                main_pair(0, (Sv, pos[("T", i)]))
                main_pair(1,
                          (Sv, pos[("L", i - 1)]) if i >= 1 else None,
                          (So, 16 * (i - 3)) if i >= 4 else None)
                main_pair(2, (Sa, apos[("sq", i, 0)]))      # sq0: slabA free
                sign_pair(0)                                  # covered by mm2
                main_pair(3, (Sa, apos[("sq", i, 1)]))      # sq1: slabB free
                sign_pair(1, None,
                          (Sv, pos[("R", i - 1)]) if i >= 1 else None)
                sign_pair(2, (Sa, apos[("sq", i, 2)]))      # sq2
                sign_pair(3, (Sa, apos[("sq", i, 3)]))      # sq3

    return nc


_CACHE = {}


def _get_nc():
    if "nc" not in _CACHE:
        _CACHE["nc"] = _build_bass()
    return _CACHE["nc"], None


# ---------------------------------------------------------------- entry point
def _make_in_maps(x, weights, scale):
    x = np.asarray(x, dtype=np.float32)
    ccp, sg2 = _host_matrices(weights)
    ws = np.zeros((128, 134), ml_dtypes.bfloat16)
    ws[0:64, 0:128] = ccp.astype(ml_dtypes.bfloat16)
    ws[64:128, 0:128] = ccp.astype(ml_dtypes.bfloat16)
    ws[:, 128:134] = sg2.astype(ml_dtypes.bfloat16)

    hs = 0.5 * float(np.asarray(scale).reshape(-1)[0])
    a = x * hs                                   # (B, 6) half-angles
    L = np.arange(128)
    wbias = np.where(((L >> 5) & 1) == 0, math.pi / 2, 0.0).astype(np.float32)
    in_maps = []
    for k in range(NCORES):
        ak = a[k * BC:(k + 1) * BC]              # (4096, 6)
        lane = ak[_SL].reshape(128, 384)
        xs = np.empty((128, 832), np.float32)
        xs[:, 0:384] = lane + np.float32(math.pi / 2)
        xs[:, 384:768] = lane
        xs[:, 768:832] = lane[:, 5::6] + wbias[:, None]
        in_maps.append({"xin": xs, "win": ws})
    return in_maps


def kernel(x, weights, scale):
    nc, _ = _get_nc()
    in_maps = _make_in_maps(x, weights, scale)
    res = run_bass_kernel_spmd(nc, in_maps, list(range(NCORES))).results
    ev = np.empty((B, NQ), np.float32)
    for k in range(NCORES):
        r = np.asarray(res[k]["out"]).astype(np.float32)   # (38, 2048)
        for h in range(2):
            for rb in range(2):                 # row-block = m//1024
                chunk = r[32 * rb:32 * rb + NQ, 1024 * h:1024 * h + 1024]
                samp = _SAMP[h, 1024 * rb:1024 * rb + 1024]
                ev[k * BC + samp, :] = chunk.T
    return ev


if __name__ == "__main__":
    rng = np.random.default_rng(0)
    x = rng.standard_normal((B, NQ)).astype(np.float32)
    weights = rng.uniform(0, 2 * np.pi, (NL, NQ, 3)).astype(np.float32)
    scale = np.array([0.1], np.float32)
    ev = kernel(x, weights, scale)
    print("out", ev.shape, ev.dtype, ev[:2])


# revision 18
# speedup vs baseline: 1.0728x; 1.0728x over previous
"""Trainium2 Bass kernel for nn_EnhancedQuantumLayer (6-qubit circuit, B=32768).

Reduction: the circuit is AngleEmbedding (per-sample RX product state) followed
by a batch-independent 64x64 unitary U (StronglyEntanglingLayers + CNOT rings,
function of `weights` only), then per-qubit PauliZ expectations.

Per sample b:
    m_b   = kron_q [cos(a_q/2), sin(a_q/2)]           (real 64-vec, a = x*scale)
    A_b   = [Re(Cc^T) ; Im(Cc^T)] m_b                 (128-vec; Cc folds the
                                                       (-i)^popcount embedding
                                                       phases into U)
    EV_bq = sum_p sgn2[p,q] * A_b[p]^2                (signs of PauliZ)

Device work per core (4096 samples), bf16 matmul pipeline:
    SP    1 input DMA/rep (angles f32, 426KB) + 1 bf16 output DMA/rep
          (156KB, issued 3 reps late so its wait is always stale);
          weights (bf16, 34KB) DMA'd once outside the rep loop
    ACT   1 fused Sin (832 cols, bf16 out) scheduled 3 reps AHEAD of its
          rep + 4 Squares (PSUM->bf16 SBUF); Sin/Square share one
          activation table (trig_and_small) -> no table reloads
    Pool  4 broadcast-AP kron multiplies (k1,k2,k3,m12; SBUF only)
    DVE   last kron stage (mswz, bf16), bf16 StreamTranspose (= M_T,
          basis on partitions; 2x 16-bit mode), and the EV copy split
          into 2 (38,1024) PSUM->SBUF bf16 halves, emitted 2 reps late
          so the transpose never queues behind the previous rep's tail
    PE    8 bf16 64->128 main matmuls + 8 bf16 sign matmuls (512-col
          chunks; PSUM-bank limit), signs interleaved with mains; sign
          outputs packed at partition bases 0/32 of a dedicated evp
          bank pair

xt/scs x4-buffered, all other SBUF streams x2-x4; PSUM: 2 A-slabs
(2 banks each) + evp (4 banks). Host does the tiny weights->matrix
precompute, the lane permutation/bias pre-add of the input, and the
inverse permutation of the bf16 output.
"""
import math
from contextlib import ExitStack

import numpy as np
import ml_dtypes

import concourse.bass as bass
import concourse.mybir as mybir
from concourse.bass_utils import run_bass_kernel_spmd

F32 = mybir.dt.float32
BF16 = mybir.dt.bfloat16
NQ = 6
NL = 6
B = 32768
NCORES = 8
BC = B // NCORES          # 4096 samples per core
NSB = 64                  # angle blocks per lane (s, t', p_hi)


# ---------------------------------------------------------------- host precompute
def _host_matrices(weights):
    """(CcPacked (64,128) f32, SgnZ2 (128,6) f32) from weights (6,6,3)."""
    w = np.asarray(weights, dtype=np.float64)
    phi, theta, omega = w[..., 0], w[..., 1], w[..., 2]
    ct, st = np.cos(0.5 * theta), np.sin(0.5 * theta)
    em = np.exp(-0.5j * (phi + omega))
    ep = np.exp(0.5j * (phi + omega))
    epm = np.exp(0.5j * (phi - omega))
    emp = np.exp(-0.5j * (phi - omega))

    state = np.eye(64, dtype=np.complex128).reshape((64,) + (2,) * NQ)

    def apply_1q(state, U, q):
        ax = q + 1
        s = np.moveaxis(state, ax, -1)
        s = np.einsum('ij,...j->...i', U, s)
        return np.moveaxis(s, -1, ax)

    def cnot(state, c, t):
        ca, ta = c + 1, t + 1
        s0 = np.take(state, 0, axis=ca)
        s1 = np.take(state, 1, axis=ca)
        t_in = ta - 1 if ta > ca else ta
        s1 = np.flip(s1, axis=t_in)
        return np.stack([s0, s1], axis=ca)

    for l in range(NL):
        for q in range(NQ):
            U = np.array([
                [em[l, q] * ct[l, q], -epm[l, q] * st[l, q]],
                [emp[l, q] * st[l, q], ep[l, q] * ct[l, q]],
            ])
            state = apply_1q(state, U, q)
        r = (l % (NQ - 1)) + 1
        for q in range(NQ):
            state = cnot(state, q, (q + r) % NQ)

    stateF = state.reshape(64, 64)            # [in_e, out_o] = U[o, e]
    e = np.arange(64)
    pc = np.array([bin(v).count('1') for v in e])
    phase = (-1j) ** pc                       # (-i)^popcount: RX embedding phases
    Cc = phase[:, None] * stateF              # (64_in, 64_out)

    # device row j has qubit q at bit q; reference index e has qubit 0 as MSB
    bitrev = np.array([int(format(j, '06b')[::-1], 2) for j in range(64)])
    Cdev = Cc[bitrev, :]

    ccpacked = np.concatenate([Cdev.real, Cdev.imag], axis=1)   # (64, 128)

    o = np.arange(64)
    z = np.stack([1.0 - 2.0 * ((o >> (5 - q)) & 1) for q in range(NQ)], axis=1)
    sgn2 = np.concatenate([z, z], axis=0)                        # (128, 6)
    return ccpacked.astype(np.float32), sgn2.astype(np.float32)


def _lane_sample_index():
    """SL[L, sb]: sample_local for lane L, angle-block sb."""
    L = np.arange(128)
    h, jh, pl = L >> 6, (L >> 5) & 1, L & 31
    sb = np.arange(64)
    s, tp, p_hi = sb >> 4, (sb >> 2) & 3, sb & 3
    return (1024 * p_hi[None, :] + 32 * pl[:, None]
            + 8 * s[None, :] + 2 * tp[None, :] + h[:, None])


def _out_sample_index():
    """SAMP[h, m]: sample_local for group h, M_T column m (m = 32*sb + pl)."""
    h = np.arange(2)[:, None]
    m = np.arange(2048)[None, :]
    sb, pl = m >> 5, m & 31
    s, tp, p_hi = sb >> 4, (sb >> 2) & 3, sb & 3
    return 1024 * p_hi + 32 * pl + 8 * s + 2 * tp + h


_SL = _lane_sample_index()
_SAMP = _out_sample_index()


# ---------------------------------------------------------------- device program
def _build_bass(reps=1):
    nc = bass.Bass()
    xin = nc.dram_tensor("xin", [128, 832], F32, kind="ExternalInput")
    win = nc.dram_tensor("win", [128, 134], BF16, kind="ExternalInput")
    out = nc.dram_tensor("out", [38, 2048], BF16, kind="ExternalOutput")

    ctx = ExitStack()
    with ctx:
        sb2 = lambda nm, shape, dt: [
            ctx.enter_context(nc.sbuf_tensor(f"{nm}{p}", shape, dt))
            for p in range(2)]
        ps = lambda nm, shape: ctx.enter_context(nc.psum_tensor(nm, shape, F32))

        xt = [ctx.enter_context(nc.sbuf_tensor(f"xt{p}", [128, 832], F32))
              for p in range(4)]
        scs = [ctx.enter_context(nc.sbuf_tensor(f"scs{p}", [128, 832], BF16))
               for p in range(4)]
        k1b = sb2("k1b", [128, 256], F32)
        k2b = sb2("k2b", [128, 256], F32)
        k3b = sb2("k3b", [128, 128], BF16)
        m12b = sb2("m12b", [128, 1024], BF16)
        mswz = sb2("mswz", [128, 2048], BF16)
        mtall = sb2("mtall", [128, 2048], BF16)
        sq = sb2("sq", [128, 4096], BF16)
        stg = [ctx.enter_context(nc.sbuf_tensor(f"stg{p}", [38, 2048], BF16))
               for p in range(4)]
        wt = ctx.enter_context(nc.sbuf_tensor("wt", [128, 134], BF16))

        slab = [ps("slabA", [128, 1024]), ps("slabB", [128, 1024])]
        evp = ps("evp", [38, 2048])

        sem = lambda nm: ctx.enter_context(nc.semaphore(name=nm))
        Sd, Sa, Sk, Sv, Sp, So = (sem("Sd"), sem("Sa"), sem("Sk"),
                                  sem("Sv"), sem("Sp"), sem("So"))

        block = ctx.enter_context(nc.Block())

        def hsq(P, q):
            return (scs[P].ap()[:, 0:768]
                    .rearrange("p (hf sb q) -> p sb hf q", hf=2, q=NQ)
                    [:, :, :, q:q + 1])

        # --- DVE emission order (Sv positions), computed up front
        dve_seq = []
        for i in range(reps):
            if i >= 2:
                dve_seq += [("L", i - 2), ("R", i - 2)]
            else:
                dve_seq += [("dL", i), ("dR", i)]
            dve_seq += [("K5", i), ("T", i)]
        for r in range(max(0, reps - 2), reps):
            dve_seq += [("L", r), ("R", r)]
        pos = {key: idx + 1 for idx, key in enumerate(dve_seq)}

        @block.sync
        def _(sync):
            sync.dma_start(out=wt.ap()[:, :], in_=win[:, :]).then_inc(Sd, 16)
            done = set()

            def out_dma(r):
                o = sync.dma_start(out=out[:, :], in_=stg[r % 4].ap()[:, :])
                o._wait_ge(Sv, pos[("R", r)]).then_inc(So, 16)
                done.add(r)

            for i in range(reps):
                d = sync.dma_start(out=xt[i % 4].ap()[:, :], in_=xin[:, :])
                if i >= 4:
                    d._wait_ge(Sk, 4 * i - 13)  # kron3(i-4): scs+xt free
                d.then_inc(Sd, 16)
                if i >= 3:
                    out_dma(i - 3)
            for r in range(reps):
                if r not in done:
                    out_dma(r)
            sync.wait_ge(So, 16 * reps)

        # --- ACT emission order (Sa positions): Sin runs 3 reps ahead
        act_seq = [("sin", k) for k in range(min(3, reps))]
        for i in range(reps):
            act_seq += [("sq", i, j) for j in range(4)]
            if i + 3 < reps:
                act_seq.append(("sin", i + 3))
        apos = {key: idx + 1 for idx, key in enumerate(act_seq)}

        @block.scalar
        def _(scalar):
            sfn = mybir.ActivationFunctionType.Sin
            sqf = mybir.ActivationFunctionType.Square
            for key in act_seq:
                if key[0] == "sin":
                    i = key[1]
                    a = nc.scalar.activation(scs[i % 4].ap()[:, :],
                                             xt[i % 4].ap()[:, :], sfn)
                    a._wait_ge(Sd, 16 * (i + 2)).then_inc(Sa, 1)
                else:
                    _, i, j = key
                    thr = {0: 2, 1: 4, 2: 6, 3: 10}[j]
                    q_ = nc.scalar.activation(
                        sq[i % 2].ap()[:, 1024 * j:1024 * j + 1024],
                        slab[j % 2].ap()[:, :], sqf)
                    q_._wait_ge(Sp, 16 * i + thr).then_inc(Sa, 1)

        @block.gpsimd
        def _(g):
            # Sk: 4/rep (4 krons; K5 lives on DVE)
            for i in range(reps):
                P = i % 2
                P4 = i % 4
                o1 = k1b[P].ap().rearrange("p (sb b1 b0) -> p sb b1 b0",
                                           b1=2, b0=2)
                i0 = hsq(P4, 0).squeeze(3).unsqueeze(2).broadcast_to((128, 64, 2, 2))
                i1 = hsq(P4, 1).squeeze(3).unsqueeze(3).broadcast_to((128, 64, 2, 2))
                t = nc.gpsimd.tensor_mul(o1, i0, i1)
                t._wait_ge(Sa, apos[("sin", i)]).then_inc(Sk, 1)
                o2 = k2b[P].ap().rearrange("p (sb b3 b2) -> p sb b3 b2",
                                           b3=2, b2=2)
                i0 = hsq(P4, 2).squeeze(3).unsqueeze(2).broadcast_to((128, 64, 2, 2))
                i1 = hsq(P4, 3).squeeze(3).unsqueeze(3).broadcast_to((128, 64, 2, 2))
                t = nc.gpsimd.tensor_mul(o2, i0, i1)
                if i >= 2:
                    t._wait_ge(Sv, pos[("K5", i - 2)])  # m12b/k3b[P] free
                t.then_inc(Sk, 1)
                o3 = k3b[P].ap().rearrange("p (sb b4) -> p sb b4", b4=2)
                i0 = hsq(P4, 4).squeeze(3)
                i1 = (scs[P4].ap()[:, 768:832]
                      .rearrange("p (sb o) -> p sb o", o=1)
                      .broadcast_to((128, 64, 2)))
                nc.gpsimd.tensor_mul(o3, i0, i1).then_inc(Sk, 1)
                om = m12b[P].ap().rearrange("p (sb b32 b10) -> p sb b32 b10",
                                            b32=4, b10=4)
                i0 = (k1b[P].ap().rearrange("p (sb w) -> p sb w", w=4)
                      .unsqueeze(2).broadcast_to((128, 64, 4, 4)))
                i1 = (k2b[P].ap().rearrange("p (sb w) -> p sb w", w=4)
                      .unsqueeze(3).broadcast_to((128, 64, 4, 4)))
                nc.gpsimd.tensor_mul(om, i0, i1).then_inc(Sk, 1)

        @block.vector
        def _(v):
            for key in dve_seq:
                kind, r = key
                P, Q = r % 2, r % 4
                if kind == "L":
                    c = nc.vector.tensor_copy(stg[Q].ap()[:, 0:1024],
                                              evp.ap()[:, 0:1024])
                    c._wait_ge(Sp, 16 * r + 12).then_inc(Sv, 1)
                elif kind == "R":
                    c = nc.vector.tensor_copy(stg[Q].ap()[:, 1024:2048],
                                              evp.ap()[:, 1024:2048])
                    c._wait_ge(Sp, 16 * r + 16).then_inc(Sv, 1)
                elif kind in ("dL", "dR"):
                    dm = nc.vector.tensor_copy(stg[Q].ap()[0:1, 0:4],
                                               stg[Q].ap()[0:1, 4:8])
                    dm.then_inc(Sv, 1)
                elif kind == "K5":
                    k5 = nc.vector.tensor_mul(
                        mswz[P].ap().rearrange("p (sb b4 w) -> p sb b4 w",
                                               b4=2, w=16),
                        (m12b[P].ap().rearrange("p (sb w) -> p sb w", w=16)
                         .unsqueeze(2).broadcast_to((128, 64, 2, 16))),
                        (k3b[P].ap().rearrange("p (sb b4) -> p sb b4", b4=2)
                         .unsqueeze(3).broadcast_to((128, 64, 2, 16))))
                    k5._wait_ge(Sk, 4 * r + 4).then_inc(Sv, 1)
                else:   # T
                    t = nc.vector.transpose(mtall[P].ap()[:, :],
                                            mswz[P].ap()[:, :])
                    t.then_inc(Sv, 1)

        @block.tensor
        def _(tensor):
            # Sp: 16/rep; order: mm0p mm1p mm2p q0p mm3p q1p q2p q3p
            for i in range(reps):
                P = i % 2

                def main_pair(j, wait=None, wait1=None):
                    h, half = divmod(j, 2)
                    for k in range(2):
                        mm = nc.tensor.matmul(
                            slab[j % 2].ap()[:, 512 * k:512 * k + 512],
                            wt.ap()[64 * h:64 * h + 64, 0:128],
                            mtall[P].ap()[64 * h:64 * h + 64,
                                          1024 * half + 512 * k:
                                          1024 * half + 512 * k + 512],
                            start=True, stop=True)
                        w = wait if k == 0 else wait1
                        if w is not None:
                            mm._wait_ge(*w)
                        mm.then_inc(Sp, 1)

                def sign_pair(q, wait0=None, wait1=None):
                    for k in range(2):
                        mm = nc.tensor.matmul(
                            evp.ap()[32 * (q % 2):32 * (q % 2) + NQ,
                                     1024 * (q // 2) + 512 * k:
                                     1024 * (q // 2) + 512 * k + 512],
                            wt.ap()[:, 128:134],
                            sq[P].ap()[:, 1024 * q + 512 * k:
                                       1024 * q + 512 * k + 512],
                            start=True, stop=True)
                        w = wait0 if k == 0 else wait1
                        if w is not None:
                            mm._wait_ge(*w)
                        mm.then_inc(Sp, 1)

                main_pair(0, (Sv, pos[("T", i)]))
                main_pair(1,
                          (Sv, pos[("L", i - 1)]) if i >= 1 else None,
                          (So, 16 * (i - 3)) if i >= 4 else None)
                main_pair(2, (Sa, apos[("sq", i, 0)]))      # sq0: slabA free
                sign_pair(0)                                  # covered by mm2
                main_pair(3, (Sa, apos[("sq", i, 1)]))      # sq1: slabB free
                sign_pair(1, None,
                          (Sv, pos[("R", i - 1)]) if i >= 1 else None)
                sign_pair(2, (Sa, apos[("sq", i, 2)]))      # sq2
                sign_pair(3, (Sa, apos[("sq", i, 3)]))      # sq3

    return nc


_CACHE = {}


def _get_nc():
    if "nc" not in _CACHE:
        _CACHE["nc"] = _build_bass()
    return _CACHE["nc"], None


# ---------------------------------------------------------------- entry point
def _make_in_maps(x, weights, scale):
    x = np.asarray(x, dtype=np.float32)
    ccp, sg2 = _host_matrices(weights)
    ws = np.zeros((128, 134), ml_dtypes.bfloat16)
    ws[0:64, 0:128] = ccp.astype(ml_dtypes.bfloat16)
    ws[64:128, 0:128] = ccp.astype(ml_dtypes.bfloat16)
    ws[:, 128:134] = sg2.astype(ml_dtypes.bfloat16)

    hs = 0.5 * float(np.asarray(scale).reshape(-1)[0])
    a = x * hs                                   # (B, 6) half-angles
    L = np.arange(128)
    wbias = np.where(((L >> 5) & 1) == 0, math.pi / 2, 0.0).astype(np.float32)
    in_maps = []
    for k in range(NCORES):
        ak = a[k * BC:(k + 1) * BC]              # (4096, 6)
        lane = ak[_SL].reshape(128, 384)
        xs = np.empty((128, 832), np.float32)
        xs[:, 0:384] = lane + np.float32(math.pi / 2)
        xs[:, 384:768] = lane
        xs[:, 768:832] = lane[:, 5::6] + wbias[:, None]
        in_maps.append({"xin": xs, "win": ws})
    return in_maps


def kernel(x, weights, scale):
    nc, _ = _get_nc()
    in_maps = _make_in_maps(x, weights, scale)
    res = run_bass_kernel_spmd(nc, in_maps, list(range(NCORES))).results
    ev = np.empty((B, NQ), np.float32)
    for k in range(NCORES):
        r = np.asarray(res[k]["out"]).astype(np.float32)   # (38, 2048)
        for h in range(2):
            for rb in range(2):                 # row-block = m//1024
                chunk = r[32 * rb:32 * rb + NQ, 1024 * h:1024 * h + 1024]
                samp = _SAMP[h, 1024 * rb:1024 * rb + 1024]
                ev[k * BC + samp, :] = chunk.T
    return ev


if __name__ == "__main__":
    rng = np.random.default_rng(0)
    x = rng.standard_normal((B, NQ)).astype(np.float32)
    weights = rng.uniform(0, 2 * np.pi, (NL, NQ, 3)).astype(np.float32)
    scale = np.array([0.1], np.float32)
    ev = kernel(x, weights, scale)
    print("out", ev.shape, ev.dtype, ev[:2])


# revision 19
# speedup vs baseline: 3.0720x; 2.8635x over previous
"""Trainium2 Bass kernel for nn_EnhancedQuantumLayer (6-qubit circuit, B=32768).

Reduction: the circuit is AngleEmbedding (per-sample RX product state) followed
by a batch-independent 64x64 unitary U (StronglyEntanglingLayers + CNOT rings,
function of `weights` only), then per-qubit PauliZ expectations.

Per sample b:
    m_b   = kron_q [cos(a_q/2), sin(a_q/2)]           (real 64-vec, a = x*scale)
    A_b   = [Re(Cc^T) ; Im(Cc^T)] m_b                 (128-vec; Cc folds the
                                                       (-i)^popcount embedding
                                                       phases into U)
    EV_bq = sum_p sgn2[p,q] * A_b[p]^2                (signs of PauliZ)

Device work per core (4096 samples), bf16 matmul pipeline:
    SP    1 input DMA (angles f32, 426KB) + 1 bf16 output DMA (156KB);
          weights (bf16, 34KB) DMA'd once outside the rep loop
    ACT   1 fused Sin (832 cols, bf16 out) + 4 Square (PSUM->bf16 SBUF);
          sin/square/copy share one activation table (trig_and_small)
    Pool  5 fused broadcast-AP kron multiplies building M in the 32x32
          block-swizzled layout (SBUF only; Pool cannot touch PSUM)
    DVE   1 bf16 StreamTranspose (= M_T, basis on partitions) + 1 fat
          (38,2048) PSUM->SBUF bf16 EV copy
    PE    8 bf16 64->128 main matmuls + 8 bf16 sign matmuls (512-col
          chunks; PSUM-bank limit), sign outputs packed at partition
          bases 0/32 of a dedicated evp bank pair

All SBUF buffers double-buffered; PSUM: 2 A-slabs (2 banks each) + evp
(4 banks). Host does the tiny weights->matrix precompute, the lane
permutation/bias pre-add of the input, and the inverse permutation of
the bf16 output.
"""
import math
from contextlib import ExitStack

import numpy as np
import ml_dtypes

import concourse.bass as bass
import concourse.mybir as mybir
from concourse.bass_utils import run_bass_kernel_spmd

F32 = mybir.dt.float32
BF16 = mybir.dt.bfloat16
NQ = 6
NL = 6
B = 32768
NCORES = 8
BC = B // NCORES          # 4096 samples per core
NSB = 64                  # angle blocks per lane (s, t', p_hi)


# ---------------------------------------------------------------- host precompute
def _host_matrices(weights):
    """(CcPacked (64,128) f32, SgnZ2 (128,6) f32) from weights (6,6,3)."""
    w = np.asarray(weights, dtype=np.float64)
    phi, theta, omega = w[..., 0], w[..., 1], w[..., 2]
    ct, st = np.cos(0.5 * theta), np.sin(0.5 * theta)
    em = np.exp(-0.5j * (phi + omega))
    ep = np.exp(0.5j * (phi + omega))
    epm = np.exp(0.5j * (phi - omega))
    emp = np.exp(-0.5j * (phi - omega))

    state = np.eye(64, dtype=np.complex128).reshape((64,) + (2,) * NQ)

    def apply_1q(state, U, q):
        ax = q + 1
        s = np.moveaxis(state, ax, -1)
        s = np.einsum('ij,...j->...i', U, s)
        return np.moveaxis(s, -1, ax)

    def cnot(state, c, t):
        ca, ta = c + 1, t + 1
        s0 = np.take(state, 0, axis=ca)
        s1 = np.take(state, 1, axis=ca)
        t_in = ta - 1 if ta > ca else ta
        s1 = np.flip(s1, axis=t_in)
        return np.stack([s0, s1], axis=ca)

    for l in range(NL):
        for q in range(NQ):
            U = np.array([
                [em[l, q] * ct[l, q], -epm[l, q] * st[l, q]],
                [emp[l, q] * st[l, q], ep[l, q] * ct[l, q]],
            ])
            state = apply_1q(state, U, q)
        r = (l % (NQ - 1)) + 1
        for q in range(NQ):
            state = cnot(state, q, (q + r) % NQ)

    stateF = state.reshape(64, 64)            # [in_e, out_o] = U[o, e]
    e = np.arange(64)
    pc = np.array([bin(v).count('1') for v in e])
    phase = (-1j) ** pc                       # (-i)^popcount: RX embedding phases
    Cc = phase[:, None] * stateF              # (64_in, 64_out)

    # device row j has qubit q at bit q; reference index e has qubit 0 as MSB
    bitrev = np.array([int(format(j, '06b')[::-1], 2) for j in range(64)])
    Cdev = Cc[bitrev, :]

    ccpacked = np.concatenate([Cdev.real, Cdev.imag], axis=1)   # (64, 128)

    o = np.arange(64)
    z = np.stack([1.0 - 2.0 * ((o >> (5 - q)) & 1) for q in range(NQ)], axis=1)
    sgn2 = np.concatenate([z, z], axis=0)                        # (128, 6)
    return ccpacked.astype(np.float32), sgn2.astype(np.float32)


def _lane_sample_index():
    """SL[L, sb]: sample_local for lane L, angle-block sb."""
    L = np.arange(128)
    h, jh, pl = L >> 6, (L >> 5) & 1, L & 31
    sb = np.arange(64)
    s, tp, p_hi = sb >> 4, (sb >> 2) & 3, sb & 3
    return (1024 * p_hi[None, :] + 32 * pl[:, None]
            + 8 * s[None, :] + 2 * tp[None, :] + h[:, None])


def _out_sample_index():
    """SAMP[h, m]: sample_local for group h, M_T column m (m = 32*sb + pl)."""
    h = np.arange(2)[:, None]
    m = np.arange(2048)[None, :]
    sb, pl = m >> 5, m & 31
    s, tp, p_hi = sb >> 4, (sb >> 2) & 3, sb & 3
    return 1024 * p_hi + 32 * pl + 8 * s + 2 * tp + h


_SL = _lane_sample_index()
_SAMP = _out_sample_index()


# ---------------------------------------------------------------- device program
def _build_bass(reps=1):
    nc = bass.Bass()
    xin = nc.dram_tensor("xin", [128, 832], F32, kind="ExternalInput")
    win = nc.dram_tensor("win", [128, 134], BF16, kind="ExternalInput")
    out = nc.dram_tensor("out", [38, 2048], BF16, kind="ExternalOutput")

    ctx = ExitStack()
    with ctx:
        sb2 = lambda nm, shape, dt: [
            ctx.enter_context(nc.sbuf_tensor(f"{nm}{p}", shape, dt))
            for p in range(2)]
        ps = lambda nm, shape: ctx.enter_context(nc.psum_tensor(nm, shape, F32))

        xt = [ctx.enter_context(nc.sbuf_tensor(f"xt{p}", [128, 832], F32))
              for p in range(4)]
        scs = [ctx.enter_context(nc.sbuf_tensor(f"scs{p}", [128, 832], BF16))
               for p in range(4)]
        k1b = sb2("k1b", [128, 256], F32)
        k2b = sb2("k2b", [128, 256], F32)
        k3b = sb2("k3b", [128, 128], BF16)
        m12b = sb2("m12b", [128, 1024], BF16)
        mswz = sb2("mswz", [128, 2048], BF16)
        mtall = sb2("mtall", [128, 2048], BF16)
        sq = sb2("sq", [128, 4096], BF16)
        stg = [ctx.enter_context(nc.sbuf_tensor(f"stg{p}", [38, 2048], BF16))
               for p in range(4)]
        wt = ctx.enter_context(nc.sbuf_tensor("wt", [128, 134], BF16))

        slab = [ps("slabA", [128, 1024]), ps("slabB", [128, 1024])]
        evp = ps("evp", [38, 2048])

        sem = lambda nm: ctx.enter_context(nc.semaphore(name=nm))
        Sd, Sa, Sk, Sv, Sp, So = (sem("Sd"), sem("Sa"), sem("Sk"),
                                  sem("Sv"), sem("Sp"), sem("So"))

        block = ctx.enter_context(nc.Block())

        def hsq(P, q):
            return (scs[P].ap()[:, 0:768]
                    .rearrange("p (hf sb q) -> p sb hf q", hf=2, q=NQ)
                    [:, :, :, q:q + 1])

        # --- DVE emission order (Sv positions), computed up front
        dve_seq = []
        for i in range(reps):
            if i >= 2:
                dve_seq += [("L", i - 2), ("R", i - 2)]
            else:
                dve_seq += [("dL", i), ("dR", i)]
            dve_seq += [("K5", i), ("T", i)]
        for r in range(max(0, reps - 2), reps):
            dve_seq += [("L", r), ("R", r)]
        pos = {key: idx + 1 for idx, key in enumerate(dve_seq)}

        @block.sync
        def _(sync):
            sync.dma_start(out=wt.ap()[:, :], in_=win[:, :]).then_inc(Sd, 16)
            done = set()

            def out_dma(r):
                o = sync.dma_start(out=out[:, :], in_=stg[r % 4].ap()[:, :])
                o._wait_ge(Sv, pos[("R", r)]).then_inc(So, 16)
                done.add(r)

            for i in range(reps):
                d = sync.dma_start(out=xt[i % 4].ap()[:, :], in_=xin[:, :])
                if i >= 4:
                    d._wait_ge(Sk, 5 * i - 17)  # kron3(i-4): scs+xt free
                d.then_inc(Sd, 16)
                if i >= 3:
                    out_dma(i - 3)
            for r in range(reps):
                if r not in done:
                    out_dma(r)
            sync.wait_ge(So, 16 * reps)

        # --- ACT emission order (Sa positions): Sin runs 3 reps ahead
        act_seq = [("sin", min(k, reps - 1)) for k in range(min(3, reps))]
        act_seq = [("sin", k) for k in range(min(3, reps))]
        for i in range(reps):
            act_seq += [("sq", i, j) for j in range(4)]
            if i + 3 < reps:
                act_seq.append(("sin", i + 3))
        apos = {key: idx + 1 for idx, key in enumerate(act_seq)}

        @block.scalar
        def _(scalar):
            sfn = mybir.ActivationFunctionType.Sin
            sqf = mybir.ActivationFunctionType.Square
            for key in act_seq:
                if key[0] == "sin":
                    i = key[1]
                    a = nc.scalar.activation(scs[i % 4].ap()[:, :],
                                             xt[i % 4].ap()[:, :], sfn)
                    a._wait_ge(Sd, 16 * (i + 2)).then_inc(Sa, 1)
                else:
                    _, i, j = key
                    thr = {0: 2, 1: 4, 2: 6, 3: 10}[j]
                    q_ = nc.scalar.activation(
                        sq[i % 2].ap()[:, 1024 * j:1024 * j + 1024],
                        slab[j % 2].ap()[:, :], sqf)
                    q_._wait_ge(Sp, 16 * i + thr).then_inc(Sa, 1)

        @block.gpsimd
        def _(g):
            # Sk: 5/rep (kron1-4 + K5a; K5b lives on DVE)
            for i in range(reps):
                P = i % 2
                P4 = i % 4
                o1 = k1b[P].ap().rearrange("p (sb b1 b0) -> p sb b1 b0",
                                           b1=2, b0=2)
                i0 = hsq(P4, 0).squeeze(3).unsqueeze(2).broadcast_to((128, 64, 2, 2))
                i1 = hsq(P4, 1).squeeze(3).unsqueeze(3).broadcast_to((128, 64, 2, 2))
                t = nc.gpsimd.tensor_mul(o1, i0, i1)
                t._wait_ge(Sa, apos[("sin", i)]).then_inc(Sk, 1)
                o2 = k2b[P].ap().rearrange("p (sb b3 b2) -> p sb b3 b2",
                                           b3=2, b2=2)
                i0 = hsq(P4, 2).squeeze(3).unsqueeze(2).broadcast_to((128, 64, 2, 2))
                i1 = hsq(P4, 3).squeeze(3).unsqueeze(3).broadcast_to((128, 64, 2, 2))
                t = nc.gpsimd.tensor_mul(o2, i0, i1)
                if i >= 2:
                    t._wait_ge(Sv, pos[("K5", i - 2)])  # m12b/k3b[P] free
                t.then_inc(Sk, 1)
                o3 = k3b[P].ap().rearrange("p (sb b4) -> p sb b4", b4=2)
                i0 = hsq(P4, 4).squeeze(3)
                i1 = (scs[P4].ap()[:, 768:832]
                      .rearrange("p (sb o) -> p sb o", o=1)
                      .broadcast_to((128, 64, 2)))
                nc.gpsimd.tensor_mul(o3, i0, i1).then_inc(Sk, 1)
                om = m12b[P].ap().rearrange("p (sb b32 b10) -> p sb b32 b10",
                                            b32=4, b10=4)
                i0 = (k1b[P].ap().rearrange("p (sb w) -> p sb w", w=4)
                      .unsqueeze(2).broadcast_to((128, 64, 4, 4)))
                i1 = (k2b[P].ap().rearrange("p (sb w) -> p sb w", w=4)
                      .unsqueeze(3).broadcast_to((128, 64, 4, 4)))
                nc.gpsimd.tensor_mul(om, i0, i1).then_inc(Sk, 1)
                # K5a: first 24 sb-blocks of the last kron on Pool
                oMa = (mswz[P].ap()[:, 0:768]
                       .rearrange("p (sb b4 w) -> p sb b4 w", b4=2, w=16))
                i0a = (m12b[P].ap()[:, 0:384]
                       .rearrange("p (sb w) -> p sb w", w=16)
                       .unsqueeze(2).broadcast_to((128, 24, 2, 16)))
                i1a = (k3b[P].ap()[:, 0:48]
                       .rearrange("p (sb b4) -> p sb b4", b4=2)
                       .unsqueeze(3).broadcast_to((128, 24, 2, 16)))
                ka = nc.gpsimd.tensor_mul(oMa, i0a, i1a)
                if i >= 2:
                    ka._wait_ge(Sv, pos[("T", i - 2)])   # mswz[P] free
                ka.then_inc(Sk, 1)

        @block.vector
        def _(v):
            for key in dve_seq:
                kind, r = key
                P, Q = r % 2, r % 4
                if kind == "L":
                    c = nc.vector.tensor_copy(stg[Q].ap()[:, 0:1024],
                                              evp.ap()[:, 0:1024])
                    c._wait_ge(Sp, 16 * r + 12).then_inc(Sv, 1)
                elif kind == "R":
                    c = nc.vector.tensor_copy(stg[Q].ap()[:, 1024:2048],
                                              evp.ap()[:, 1024:2048])
                    c._wait_ge(Sp, 16 * r + 16).then_inc(Sv, 1)
                elif kind in ("dL", "dR"):
                    dm = nc.vector.tensor_copy(stg[Q].ap()[0:1, 0:4],
                                               stg[Q].ap()[0:1, 4:8])
                    dm.then_inc(Sv, 1)
                elif kind == "K5":
                    k5 = nc.vector.tensor_mul(
                        mswz[P].ap()[:, 768:2048]
                        .rearrange("p (sb b4 w) -> p sb b4 w", b4=2, w=16),
                        (m12b[P].ap()[:, 384:1024]
                         .rearrange("p (sb w) -> p sb w", w=16)
                         .unsqueeze(2).broadcast_to((128, 40, 2, 16))),
                        (k3b[P].ap()[:, 48:128]
                         .rearrange("p (sb b4) -> p sb b4", b4=2)
                         .unsqueeze(3).broadcast_to((128, 40, 2, 16))))
                    k5._wait_ge(Sk, 5 * r + 4).then_inc(Sv, 1)
                else:   # T
                    t = nc.vector.transpose(mtall[P].ap()[:, :],
                                            mswz[P].ap()[:, :])
                    t._wait_ge(Sk, 5 * r + 5).then_inc(Sv, 1)

        @block.tensor
        def _(tensor):
            # Sp: 16/rep; order: mm0p mm1p mm2p q0p mm3p q1p q2p q3p
            for i in range(reps):
                P = i % 2

                def main_pair(j, wait=None, wait1=None):
                    h, half = divmod(j, 2)
                    for k in range(2):
                        mm = nc.tensor.matmul(
                            slab[j % 2].ap()[:, 512 * k:512 * k + 512],
                            wt.ap()[64 * h:64 * h + 64, 0:128],
                            mtall[P].ap()[64 * h:64 * h + 64,
                                          1024 * half + 512 * k:
                                          1024 * half + 512 * k + 512],
                            start=True, stop=True)
                        w = wait if k == 0 else wait1
                        if w is not None:
                            mm._wait_ge(*w)
                        mm.then_inc(Sp, 1)

                def sign_pair(q, wait0=None, wait1=None):
                    for k in range(2):
                        mm = nc.tensor.matmul(
                            evp.ap()[32 * (q % 2):32 * (q % 2) + NQ,
                                     1024 * (q // 2) + 512 * k:
                                     1024 * (q // 2) + 512 * k + 512],
                            wt.ap()[:, 128:134],
                            sq[P].ap()[:, 1024 * q + 512 * k:
                                       1024 * q + 512 * k + 512],
                            start=True, stop=True)
                        w = wait0 if k == 0 else wait1
                        if w is not None:
                            mm._wait_ge(*w)
                        mm.then_inc(Sp, 1)

                main_pair(0, (Sv, pos[("T", i)]))
                main_pair(1,
                          (Sv, pos[("L", i - 1)]) if i >= 1 else None,
                          (So, 16 * (i - 3)) if i >= 4 else None)
                main_pair(2, (Sa, apos[("sq", i, 0)]))      # sq0: slabA free
                sign_pair(0)                                  # covered by mm2
                main_pair(3, (Sa, apos[("sq", i, 1)]))      # sq1: slabB free
                sign_pair(1, None,
                          (Sv, pos[("R", i - 1)]) if i >= 1 else None)
                sign_pair(2, (Sa, apos[("sq", i, 2)]))      # sq2
                sign_pair(3, (Sa, apos[("sq", i, 3)]))      # sq3

    return nc


_CACHE = {}


def _get_nc():
    if "nc" not in _CACHE:
        _CACHE["nc"] = _build_bass()
    return _CACHE["nc"], None


# ---------------------------------------------------------------- entry point
def _make_in_maps(x, weights, scale):
    x = np.asarray(x, dtype=np.float32)
    ccp, sg2 = _host_matrices(weights)
    ws = np.zeros((128, 134), ml_dtypes.bfloat16)
    ws[0:64, 0:128] = ccp.astype(ml_dtypes.bfloat16)
    ws[64:128, 0:128] = ccp.astype(ml_dtypes.bfloat16)
    ws[:, 128:134] = sg2.astype(ml_dtypes.bfloat16)

    hs = 0.5 * float(np.asarray(scale).reshape(-1)[0])
    a = x * hs                                   # (B, 6) half-angles
    L = np.arange(128)
    wbias = np.where(((L >> 5) & 1) == 0, math.pi / 2, 0.0).astype(np.float32)
    in_maps = []
    for k in range(NCORES):
        ak = a[k * BC:(k + 1) * BC]              # (4096, 6)
        lane = ak[_SL].reshape(128, 384)
        xs = np.empty((128, 832), np.float32)
        xs[:, 0:384] = lane + np.float32(math.pi / 2)
        xs[:, 384:768] = lane
        xs[:, 768:832] = lane[:, 5::6] + wbias[:, None]
        in_maps.append({"xin": xs, "win": ws})
    return in_maps


def kernel(x, weights, scale):
    nc, _ = _get_nc()
    in_maps = _make_in_maps(x, weights, scale)
    res = run_bass_kernel_spmd(nc, in_maps, list(range(NCORES))).results
    ev = np.empty((B, NQ), np.float32)
    for k in range(NCORES):
        r = np.asarray(res[k]["out"]).astype(np.float32)   # (38, 2048)
        for h in range(2):
            for rb in range(2):                 # row-block = m//1024
                chunk = r[32 * rb:32 * rb + NQ, 1024 * h:1024 * h + 1024]
                samp = _SAMP[h, 1024 * rb:1024 * rb + 1024]
                ev[k * BC + samp, :] = chunk.T
    return ev


if __name__ == "__main__":
    rng = np.random.default_rng(0)
    x = rng.standard_normal((B, NQ)).astype(np.float32)
    weights = rng.uniform(0, 2 * np.pi, (NL, NQ, 3)).astype(np.float32)
    scale = np.array([0.1], np.float32)
    ev = kernel(x, weights, scale)
    print("out", ev.shape, ev.dtype, ev[:2])
